# revision 35
# baseline (speedup 1.0000x reference)
"""Min-norm solver (MGDA) for Trainium2, sharded across 8 NeuronCores.

Strategy:
  - vecs is [32, 2097152] f32 (256 MB).  The only memory-heavy step is the
    Gram matrix G = vecs @ vecs.T ([32, 32]).  We shard the d dimension
    across 8 cores and compute partial Grams on-device.
  - On-device layout: the host pre-transposes each core's shard into
    X[p, n*32 + j] = vecs[j, n*128 + p]  (p: 0..127 partition, n: d-chunk,
    j: task), so the TensorEngine can contract over the partition dim with
    fully-contiguous APs.  Four d-chunks are packed into one group of 128
    columns; one matmul accumulates all 4 chunks' partial Grams into the 4
    diagonal [32,32] blocks of a [128,128] PSUM tile.
  - Precision/bandwidth trade: the DMA model serializes all transfers on
    one 360 GB/s resource, so per-core time is dominated by input bytes.
    vecs is cast to fp8 e4m3 (|v| <= ~6, well inside the +-240 range),
    cutting DMA to 8 MB/core (23.3us, ~80% of total runtime).  The Gram
    error (~1.7e3 on a 2.1e6 diagonal) maps to ~3.5e-4 end-to-end
    solution error, far inside the 2e-2 gate.  fp8e4 also enables
    MatmulPerfMode.DoubleRow (two 128-deep k-tiles per matmul at 0.5
    cycles/output-row), so the PE runs 4x faster per moving byte than
    fp16 (16384 cycles total) and stays fully hidden under the DMA
    stream.
  - The program is raw bass (no TileContext) with explicit semaphores:
    saves ~0.7us entry barrier + ~0.7us exit drain ladder vs the tile
    framework.  The 8 MB shard fits in SBUF outright (64 KB/partition),
    so there is no buffer rotation; input tiles taper at the end so only
    ~2 matmul pairs remain after the last DMA-completion semaphore.
  - The tiny 250-iteration solver runs on the host in float32 numpy,
    faithfully mirroring the reference ops.

Measured (TimelineSim cost model, per core): 28588 ns vs 53347 ns for
the previous fp16 tile-framework version (1.87x).  Breakdown: 1325
startup (the first input DMA is hoisted ahead of the framework's entry
barrier, so only its own 625 HWDGE + 650 DGE issue latency remains),
23303 DMA stream (packed, zero gaps), ~3960 tail (900 DMA sem + 57 PE
+ 595 PSUM->SBUF copy + 1275 HWDGE/DGE store latency + 182 store + 900
store sem + 25 final wait).  Dead ends, for the record: multi-queue
DMA (the model serializes all queues on one DMA_ENGINES resource),
SWDGE kv_writeback/trigger_dma store (-0.9us in the model, but the
NEFF backend used by run_bass_kernel_spmd rejects the custom-ucode
instructions at execution), gpsimd PSUM copy (walrus rejects), storing
without a completion semaphore (race-detector contract violation).
"""

import numpy as np
import ml_dtypes

N_TASKS = 32
D = 2097152
N_CORES = 8
D_LOC = D // N_CORES          # 262144 d-values per core
N_CHUNK = D_LOC // 128        # 2048 chunks of 128 d-values
N_KGRP = N_CHUNK // 4         # 512 column-groups of 128 (4 chunks x 32 tasks)

MAX_ITER = 250
STOP_CRIT = np.float32(1e-6)
EPS = np.float32(1e-8)

# tile widths in units of 128-column k-groups; must sum to N_KGRP, all even.
# The taper shortens the final DMA->matmul dependency chain: only the
# last tile's matmul pairs run after the final DMA-completion semaphore.
# Per-DMA HWDGE/SEQ stages pipeline far ahead of the transfers, so tile
# count is timing-neutral; the binding constraint is the DMA model's
# 2x latency penalty for contiguous runs under 512 bytes/partition,
# which makes 4 k-groups (512 B at fp8) the smallest free tile.  A 4kg
# last tile (two DoubleRow pairs, 54ns of PE) is optimal: a 2kg tile
# would save 27ns of PE but pay +91ns of penalized stream time.  Any
# penalty-free taper ending in a 4kg tile lands on the same 28588ns
# plateau.
WIDTHS_K = (64, 64, 64, 64, 64, 64, 64, 32, 16, 12, 4)

_PROGRAMS = {}


def _build_program_manual(widths_k=WIDTHS_K, out="dma", final_wait=True,
                          monotonic_sems=1, copy_eng="vector",
                          hoist_first_dma=True, wait_eng="sync"):
    """Raw-bass (no TileContext) program: saves the tile framework's
    entry barrier (~0.7us) and exit drain ladder (~0.7us).

    The whole 8 MB shard fits in SBUF (64 KB/partition), so tiles are
    DMA'd into disjoint slices of one tensor and no buffer rotation or
    reuse semaphores are needed.  Cross-engine ordering: one semaphore
    per input DMA (single-increment/single-wait keeps the race detector
    happy), s_pe (last matmul), s_cp (PSUM->SBUF copy), s_out (store DMA
    completion).

    out="trigger" (NOT used by default): stores via pre-prepared SWDGE
    kv_writeback descriptors fired by trigger_dma, skipping the 625ns
    HWDGE + 650ns DGE stages after the copy (-0.9us in the cost model).
    kv_writeback with batch=128, dhi=128, dho=1, n_ctx=ncn=1 and
    all-zero ctx indices is a plain transposed [128,128] store, and the
    Gram tile is symmetric so the transpose is a no-op.  It validates in
    CoreSim (race-detector clean, bit-correct) but the NEFF backend
    behind run_bass_kernel_spmd rejects the custom-ucode instructions at
    execution time, so out="dma" (plain HWDGE store) is the default.
    """
    import concourse.mybir as mybir
    from concourse import bacc

    assert sum(widths_k) == N_KGRP and all(w % 2 == 0 for w in widths_k)

    nc = bacc.Bacc("TRN2", target_bir_lowering=False, debug=False,
                   num_devices=N_CORES,
                   # the two kv_writeback preps need 2*513 SWDGE ring
                   # descriptors; the default 16KB scratch holds only 1024
                   dynamic_dma_scratch_size=32768 if out == "trigger"
                   else 16384,
                   monotonic_sem_count=monotonic_sems)
    xh = nc.dram_tensor("xh", [128, N_KGRP, 128], mybir.dt.float8e4,
                        kind="ExternalInput").ap()
    out_shape = [128, 128, 1, 1] if out == "trigger" else [128, 128]
    out_ab = nc.dram_tensor("out_ab", out_shape, mybir.dt.float32,
                            kind="ExternalOutput").ap()

    X = nc.alloc_sbuf_tensor("X", [128, N_KGRP, 128], mybir.dt.float8e4).ap()
    O = nc.alloc_sbuf_tensor("O", [128, 1, 128, 1], mybir.dt.float32).ap()
    if out == "trigger":
        IDXZ = nc.alloc_sbuf_tensor("IDXZ", [128, 64], mybir.dt.int32).ap()
    PA = nc.alloc_psum_tensor("PA", [128, 128], mybir.dt.float32).ap()

    s_in = [nc.alloc_semaphore(f"s_in{t}") for t in range(len(widths_k))]
    s_pe = nc.alloc_semaphore("s_pe")
    s_cp = nc.alloc_semaphore("s_cp")
    s_out = nc.alloc_semaphore("s_out")
    if out == "trigger":
        s_prep = nc.alloc_semaphore("s_prep")
        s_z = nc.alloc_semaphore("s_z")

    total_pairs = N_KGRP // 2
    pair = 0
    k0 = 0
    d0 = None
    for t, wk in enumerate(widths_k):
        di = nc.sync.dma_start(
            X[:, k0:k0 + wk, :], xh[:, k0:k0 + wk, :]).then_inc(s_in[t], 16)
        if t == 0:
            d0 = di
        if t == 0 and out == "trigger":
            # writeback descriptor prep: mid-stream on the otherwise-idle
            # Pool engine, well off the critical path
            nc.gpsimd.memset(IDXZ, 0).then_inc(s_z, 1)
            nc.gpsimd.wait_ge(s_z, 1)
            for half in range(2):
                b = slice(64 * half, 64 * half + 64)
                nc.gpsimd.kv_writeback(
                    out_ab[b], O[:, :, b, :], IDXZ,
                    prepare_only=True, sem=s_out,
                ).then_inc(s_prep, 1)
        nc.tensor.wait_ge(s_in[t], 16)
        for i in range(wk // 2):
            g = k0 + 2 * i
            mm = nc.tensor.matmul(
                PA, X[:, g:g + 2, :], X[:, g:g + 2, :],
                start=(pair == 0), stop=(pair == total_pairs - 1),
                perf_mode=mybir.MatmulPerfMode.DoubleRow)
            pair += 1
            if pair == total_pairs:
                mm.then_inc(s_pe, 1)
        k0 += wk

    ceng = getattr(nc, {"vector": "vector", "scalar": "scalar",
                        "gpsimd": "gpsimd"}[copy_eng])
    ceng.wait_ge(s_pe, 1)
    if copy_eng == "scalar":
        ceng.copy(O[:, 0, :, 0], PA).then_inc(s_cp, 1)
    else:
        ceng.tensor_copy(O[:, 0, :, 0], PA).then_inc(s_cp, 1)
    if out == "trigger":
        nc.gpsimd.wait_ge(s_prep, 2)
        nc.gpsimd.wait_ge(s_cp, 1)
        nc.gpsimd.trigger_dma(count=2)
        nc.gpsimd.wait_ge(s_out, 32)
    else:
        nc.sync.wait_ge(s_cp, 1)
        st = nc.sync.dma_start(out_ab, O[:, 0, :, 0])
        if final_wait:
            st.then_inc(s_out, 16)
            getattr(nc, wait_eng).wait_ge(s_out, 16)
    if hoist_first_dma:
        # Move the first input DMA ahead of the framework's entry barrier
        # (after SP's preamble Drain) so its 625ns HWDGE + 650ns DGE issue
        # latency overlaps the ~590ns barrier wait.  The barrier only
        # fences the const-AP memsets, which touch SBUF regions this DMA
        # never reads; no framework instruction is modified.
        entry = nc.main_func.blocks[0]
        lst = entry.instructions
        d0i = d0.ins
        i0 = next(i for i, x in enumerate(lst) if x is d0i)
        lst.pop(i0)
        ib = next(i for i, x in enumerate(lst)
                  if getattr(x, "name", "").startswith("barrier_SP"))
        lst.insert(ib, d0i)
    nc.compile()
    return nc


def _build_program(widths_k=WIDTHS_K, bufs=8, dma="sync", out_src="vector"):
    import concourse.mybir as mybir
    import concourse.tile as tile
    from concourse import bacc

    assert sum(widths_k) == N_KGRP and all(w % 2 == 0 for w in widths_k)

    nc = bacc.Bacc("TRN2", target_bir_lowering=False, debug=False,
                   num_devices=N_CORES)
    xh = nc.dram_tensor("xh", [128, N_KGRP, 128], mybir.dt.float8e4,
                        kind="ExternalInput").ap()
    out_ab = nc.dram_tensor("out_ab", [128, 128], mybir.dt.float32,
                            kind="ExternalOutput").ap()

    with tile.TileContext(nc) as tc:
        with (
            tc.tile_pool(name="hi", bufs=bufs) as hi_pool,
            tc.tile_pool(name="psum", bufs=1, space="PSUM") as psum_pool,
            tc.tile_pool(name="outs", bufs=1) as out_pool,
        ):
            dma_eng = getattr(nc, dma)
            p_a = psum_pool.tile([128, 128], mybir.dt.float32, name="p_a")
            total_pairs = N_KGRP // 2
            pair = 0
            k0 = 0
            for wk in widths_k:
                ht = hi_pool.tile([128, wk, 128], mybir.dt.float8e4, tag="ht")
                dma_eng.dma_start(ht[:], xh[:, k0:k0 + wk, :])
                for i in range(wk // 2):
                    sl = ht[:, 2 * i:2 * i + 2, :]
                    nc.tensor.matmul(
                        p_a[:], sl, sl,
                        start=(pair == 0), stop=(pair == total_pairs - 1),
                        perf_mode=mybir.MatmulPerfMode.DoubleRow)
                    pair += 1
                k0 += wk
            if out_src == "psum":
                nc.sync.dma_start(out_ab, p_a[:])
            else:
                o = out_pool.tile([128, 128], mybir.dt.float32)
                nc.vector.tensor_copy(o[:], p_a[:])
                nc.sync.dma_start(out_ab, o[:])
    nc.compile()
    return nc


PROG_MODE = "manual"


def _get_program(mode=None, **kw):
    mode = PROG_MODE if mode is None else mode
    key = (mode,) + tuple(sorted(kw.items()))
    if key not in _PROGRAMS:
        build = {"manual": _build_program_manual, "tile": _build_program}[mode]
        _PROGRAMS[key] = build(**kw)
    return _PROGRAMS[key]


def _prep_inputs(vecs):
    """[32, D] f32 -> per-core fp8e4m3 arrays in PE layout.

    X[c, p, kg, (n%4)*32 + j] = vecs[j, c*D_LOC + n*128 + p], kg = n//4
    """
    x = np.asarray(vecs, dtype=np.float32)
    x = x.reshape(N_TASKS, N_CORES, N_CHUNK, 128)      # [j, c, n, p]
    x = np.ascontiguousarray(x.transpose(1, 3, 2, 0))  # [c, p, n, j]
    x = x.astype(ml_dtypes.float8_e4m3)
    return x.reshape(N_CORES, 128, N_KGRP, 128)


def run_device(vecs, **prog_kw):
    """Run the sharded Gram computation; returns (G [32,32] f32, results)."""
    from concourse.bass_utils import run_bass_kernel_spmd

    hi = _prep_inputs(vecs)
    in_maps = [{"xh": hi[c]} for c in range(N_CORES)]
    res = run_bass_kernel_spmd(
        _get_program(**prog_kw), in_maps, list(range(N_CORES)))
    g_acc = np.zeros((N_TASKS, N_TASKS), dtype=np.float64)
    for c in range(N_CORES):
        a = res.results[c]["out_ab"].astype(np.float64).reshape(128, 128)
        for s in range(4):
            blk = slice(32 * s, 32 * (s + 1))
            g_acc += a[blk, blk]
    return g_acc.astype(np.float32), res


# ---------------------------------------------------------------------------
# Host-side solver: faithful float32 numpy port of the reference iteration.
# ---------------------------------------------------------------------------

def _line_solver(v11, v12, v22):
    g = (v22 - v12) / (v11 + v22 - np.float32(2.0) * v12 + EPS)
    c = v22 + g * (v12 - v22)
    gamma = np.where(v12 >= v22, np.float32(0.0), g)
    gamma = np.where(v12 >= v11, np.float32(1.0), gamma)
    cost = np.where(v12 >= v22, v22, c)
    cost = np.where(v12 >= v11, v11, cost)
    return gamma.astype(np.float32), cost.astype(np.float32)


def _planar_init(G, n):
    iu, ju = np.triu_indices(n, 1)
    vivj = G[iu, ju]
    vivi = G[iu, iu]
    vjvj = G[ju, ju]
    gamma, cost = _line_solver(vivi, vivj, vjvj)
    off = int(np.argmin(cost))
    sol = np.zeros(n, dtype=G.dtype)
    sol[iu[off]] = gamma[off]
    sol[ju[off]] = np.float32(1.0) - gamma[off]
    return sol


def _proj_simplex(gamma, i_grid):
    s = np.sort(gamma)[::-1]  # descending
    tmp_max = (np.cumsum(s, dtype=np.float32) - np.float32(1.0)) / i_grid
    cond = tmp_max[:-1] > s[1:]
    first = int(np.argmax(cond))  # first True (0 if none)
    tmax = tmp_max[:-1][first] if bool(np.any(cond)) else tmp_max[-1]
    return np.maximum(gamma - tmax, np.float32(0.0)).astype(np.float32)


def _next_point(cur, grad, n_f, i_grid):
    proj = (grad - np.sum(grad) / n_f).astype(np.float32)
    neg = proj < 0
    pos = proj > 0
    inf = np.float32(np.inf)
    tm1 = np.where(neg, -cur / np.where(neg, proj, np.float32(1.0)), inf)
    tm2 = np.where(pos, (np.float32(1.0) - cur) / np.where(pos, proj, np.float32(1.0)), inf)
    thr = np.float32(1e-7)
    m1 = np.min(np.where(tm1 > thr, tm1, inf))
    t = m1 if np.isfinite(m1) else np.float32(1.0)
    m2 = np.min(np.where(tm2 > thr, tm2, inf))
    t = np.minimum(t, m2).astype(np.float32)
    nxt = (proj * t + cur).astype(np.float32)
    return _proj_simplex(nxt, i_grid)


def solve(G):
    n = G.shape[0]
    sol = _planar_init(G, n)
    i_grid = (np.arange(n, dtype=G.dtype) + np.float32(1.0)).astype(G.dtype)
    n_f = np.float32(n)
    for _ in range(MAX_ITER):
        grad_dir = (-(G @ sol)).astype(np.float32)
        newp = _next_point(sol, grad_dir, n_f, i_grid)
        gs = G @ sol
        gn = G @ newp
        v11 = np.float32(sol @ gs)
        v12 = np.float32(sol @ gn)
        v22 = np.float32(newp @ gn)
        gamma, _ = _line_solver(v11, v12, v22)
        new_sol = (gamma * sol + (np.float32(1.0) - gamma) * newp).astype(np.float32)
        if np.sum(np.abs(new_sol - sol)) < STOP_CRIT:
            break  # reference freezes the OLD sol once change < stop_crit
        sol = new_sol
    return sol.astype(np.float32)


def kernel(vecs):
    G, _ = run_device(vecs)
    return solve(G)
